# revision 41
# baseline (speedup 1.0000x reference)
"""CrossAttention (DFFNet) Trainium2 Bass kernel.

Shapes (hardcoded): rgb/depth [4, 256, 64, 64] f32; Wq/Wk [32, 256]; Wv [256, 256].

    q = Wq @ d + bq          [B, 32, 4096]
    k = Wk @ d + bk          [B, 32, 4096]
    v = Wv @ r + bv          [B, 256, 4096]
    scores = q^T k           [B, 4096, 4096], softmax over keys (last dim)
    feat = v @ mask^T        [B, 256, 4096]

Sharding: 8 cores = 4 batches x 2 query-halves (2048 queries each). Each core
gets full depth/rgb for its batch (keys/values span all 4096 tokens) plus its
query-half slice of depth.

Device layout: scores are computed TRANSPOSED, st[m, n] (keys m on partitions,
queries n free) so the feat matmul needs no transposes:
  - v^T[m, c] is produced directly by  r-slice^T @ Wv^T.
  - feat[c, n] = sum_m v^T[m, c] * exp(st[m, n]) / S[n].
  - softmax denominator S[n] = sum_m exp(st[m,n]) via ones-lhsT matmul.
The PE array is the bottleneck (99% busy), so the exp tiles and v^T tiles are
stored as fp8-e4m3 and the feat + sums matmuls run in DoubleRow perf mode
(two 128-deep key tiles contracted per instruction at 0.5 cycles/row), halving
their PE streaming time. exp is computed with a built-in bias of -ln(8)
(out = exp(st)/8) to center values in fp8e4's range [2^-9, 240]; the scale
cancels exactly in the softmax normalization since the denominator uses the
same scaled tiles. Scores stay bf16 (K=32, output-bound, fp8 wouldn't help).

Inputs arrive from the host pre-cast to bf16 (halves DMA bytes and removes
all on-chip f32->bf16 staging casts). Normalization: 1/S via fast reciprocal
(f32), cast to fp16, broadcast to 128 partitions via a K=1 fp16 matmul.
"""

import math

import numpy as np
import ml_dtypes

import concourse.bacc as bacc
import concourse.bass as bass
import concourse.mybir as mybir
import concourse.tile as tile
from concourse.bass_utils import run_bass_kernel_spmd

B, C, H, W = 4, 256, 64, 64
HW = H * W            # 4096
CQK = 32
P = 128
NQ = HW // 2          # 2048 queries per core
NT = 512              # query tile
N_NT = NQ // NT       # 4
MT = HW // P          # 32 key tiles
NPAIR = MT // 2       # 16 DoubleRow key-tile pairs
KC = C // P           # 2 contraction tiles for the projections

F32 = mybir.dt.float32
F16 = mybir.dt.float16
BF16 = mybir.dt.bfloat16
FP8 = mybir.dt.float8e4
AF = mybir.ActivationFunctionType
OP = mybir.AluOpType
DR = mybir.MatmulPerfMode.DoubleRow

EXP_BIAS = -math.log(8.0)   # exp(st)/8: keeps weights in fp8e4 normal range


def _emit(tc, io):
    nc = tc.nc
    # d and r arrive packed chunk-major with the two 128-row contraction
    # blocks interleaved: row block b = ch*2 + kc holds
    # orig[kc*128:(kc+1)*128, ch*1024:(ch+1)*1024]. Each SBUF tile is one
    # contiguous 256KB DRAM block, and the host->HBM upload order matches
    # the order the projections consume tiles in.
    d = io["d"].ap()          # [1024, 1024] bf16 depth (query-half rotated)
    r = io["r"].ap()          # [1024, 1024] bf16 rgb, same packing/rotation
    wqt4 = io["wqt4"].ap()    # [256, 128] bf16 = tile(Wq.T, (1,4))
    wkt4 = io["wkt4"].ap()    # [256, 128] bf16
    wvt = io["wvt"].ap()      # [256, 256] bf16 = Wv.T
    # all bias vectors packed in one tensor so ONE early DMA covers them:
    # col0 = tile(bq,4), col1 = tile(bk,4), col2/3 = bv[0:128]/bv[128:256]
    biasv = io["biasv"].ap()  # [128, 4] f32
    out = io["out"].ap()      # [256, 2048] f32

    from contextlib import ExitStack

    with ExitStack() as ctx:
        pw = ctx.enter_context(tc.tile_pool(name="weights", bufs=1))
        pin = ctx.enter_context(tc.tile_pool(name="inputs", bufs=1))
        pqk = ctx.enter_context(tc.tile_pool(name="qk", bufs=1))
        pvt = ctx.enter_context(tc.tile_pool(name="vt", bufs=1))
        pse = ctx.enter_context(tc.tile_pool(name="stexp", bufs=8))
        psmall = ctx.enter_context(tc.tile_pool(name="small", bufs=2))
        pout = ctx.enter_context(tc.tile_pool(name="outsb", bufs=4))
        ps_st = ctx.enter_context(
            tc.tile_pool(name="ps_st", bufs=2, space=bass.MemorySpace.PSUM))
        ps_feat = ctx.enter_context(
            tc.tile_pool(name="ps_feat", bufs=2, space=bass.MemorySpace.PSUM))
        ps_sums = ctx.enter_context(
            tc.tile_pool(name="ps_sums", bufs=2, space=bass.MemorySpace.PSUM))

        # ---- constants / weights (inputs already bf16: direct DMA) -----
        # The packed bias vector goes FIRST on the sync ring: it is consumed
        # by the earliest DVE ops, and issuing it late would alias its
        # completion semaphore with later big transfers (false dependency).
        bias_sb = pw.tile([P, 4], F32, tag="biasv")
        nc.sync.dma_start(bias_sb[:], biasv[:])
        bq_sb = bias_sb[:, 0:1]
        bk_sb = bias_sb[:, 1:2]
        bv_t = [bias_sb[:, 2:3], bias_sb[:, 3:4]]
        wq_t, wk_t, wv_t = [], [], []
        for kc in range(KC):
            t = pw.tile([P, P], BF16, tag=f"wq{kc}")
            nc.sync.dma_start(t[:], wqt4[kc * P:(kc + 1) * P, :])
            wq_t.append(t)
            t = pw.tile([P, P], BF16, tag=f"wk{kc}")
            nc.sync.dma_start(t[:], wkt4[kc * P:(kc + 1) * P, :])
            wk_t.append(t)
            t = pw.tile([P, C], BF16, tag=f"wv{kc}")
            nc.sync.dma_start(t[:], wvt[kc * P:(kc + 1) * P, :])
            wv_t.append(t)
        # Full-width ones lhsT: dual-fp8 LDWEIGHTS requires col_grp=0xf (all
        # 128 PE columns), so a [128,2,1] ones vector is illegal. The [128,2,128]
        # form also lands S[n] pre-broadcast on all 128 PSUM partitions, which
        # removes the separate K=1 broadcast matmul from the critical path.
        ones2 = pw.tile([P, 2, P], FP8, tag="ones2")
        nc.vector.memset(ones2[:], 1.0)
        ebias = pw.tile([P, 1], F32, tag="ebias")
        nc.vector.memset(ebias[:], EXP_BIAS)

        # ---- inputs (per-chunk tiles so projections start early) -------
        # d and r are column-rotated per core on the host so this core's
        # query tokens are d's FIRST 2048 columns (q-proj reads d chunks 0-1
        # directly; no separate dq input). Key/value order is a free
        # permutation: softmax and feat both just sum over keys.
        # DMA rings: d + r[kc1] on gpsimd (fast ring), r[kc0] on sync after
        # the weights. The scalar/Activation queue stays clear of
        # descriptors so exp activations are never queued behind DMA.
        def _tile_of(dram_ap, kc, ch, pref, eng):
            t = pin.tile([P, 1024], BF16, tag=f"{pref}{kc}_{ch}",
                         name=f"{pref}{kc}_{ch}")
            b = ch * 2 + kc
            eng.dma_start(t[:], dram_ap[b * P:(b + 1) * P, :])
            return t

        d_sb = [[None] * 4 for _ in range(KC)]
        for ch in range(4):
            for kc in range(KC):
                d_sb[kc][ch] = _tile_of(d, kc, ch, "d", nc.gpsimd)
        r_sb = [[None] * 4 for _ in range(KC)]
        for ch in range(4):
            for kc in range(KC):
                r_sb[kc][ch] = _tile_of(r, kc, ch, "r", nc.sync)

        # ---- q / k projections (4x-replicated layouts for row packing) --
        # q4h/k4q are SEPARATE tiles per 1024-column block so a score matmul
        # on block b only depends on block b's single wide bias-add, not on
        # the whole serialized bias chain (subtile deps are conservative).
        # Emission interleaves q-half0 / k-qtr0 FIRST: the first score matmul
        # needs exactly those two bias-adds, so the serial DVE chain reaches
        # them before anything else.
        q4h = [pqk.tile([P, 1024], BF16, tag=f"q4_{h}", name=f"q4_{h}")
               for h in range(2)]
        k4q = [pqk.tile([P, 1024], BF16, tag=f"k4_{q}", name=f"k4_{q}")
               for q in range(4)]

        def _proj(dst, blk, w_t, b_sb, pref):
            pp = ps_st.tile([P, 2, NT], F32, tag="stp", name=f"{pref}{blk}")
            for sub in range(2):
                for kc in range(KC):
                    nc.tensor.matmul(
                        pp[:, sub:sub + 1, :],
                        lhsT=w_t[kc][:],
                        rhs=d_sb[kc][blk][:, sub * NT:(sub + 1) * NT],
                        start=(kc == 0),
                        stop=(kc == KC - 1),
                    )
            nc.vector.tensor_scalar(dst[:], pp[:], b_sb, None, OP.add)

        _proj(q4h[0], 0, wq_t, bq_sb, "qp")
        _proj(k4q[0], 0, wk_t, bk_sb, "kp")
        _proj(q4h[1], 1, wq_t, bq_sb, "qp")
        _proj(k4q[1], 1, wk_t, bk_sb, "kp")
        _proj(k4q[2], 2, wk_t, bk_sb, "kp")
        _proj(k4q[3], 3, wk_t, bk_sb, "kp")

        def emit_scores(nt, g):
            stp = ps_st.tile([P, 2, NT], F32, tag="stp", name=f"stp{nt}_{g}")
            for j in range(2):
                mt = 2 * g + j
                nc.tensor.matmul(
                    stp[:, j:j + 1, :],
                    lhsT=k4q[mt // 8][32 * j:32 * j + 32,
                                      (mt % 8) * P:(mt % 8 + 1) * P],
                    rhs=q4h[nt // 2][32 * j:32 * j + 32,
                                     (nt % 2) * NT:(nt % 2 + 1) * NT],
                    start=True,
                    stop=True,
                    tile_position=(32 * j, 0),
                )
            se = pse.tile([P, 2, NT], FP8, tag="se", name=f"se{nt}_{g}")
            nc.scalar.activation(se[:], stp[:], AF.Exp, bias=ebias[:])
            return se

        seq = [(nt, g) for nt in range(N_NT) for g in range(NPAIR)]
        se_q = {}

        # ---- bank phase: v^T projection interleaved with the first LOOK
        # score/exp pairs. vtp[g][p, i, c] = v[c, (2g+i)*128 + p], stored fp8
        # in DoubleRow pair layout (no bias; bias added at the end). The vp
        # PSUM tiles rotate through the fc + sums banks (idle until the main
        # loop). Casts for the first half go to ACT (interleaving between the
        # banked exps); the second half to DVE (after the k/q bias adds) so
        # both finish just before the fc/sm allocations need the banks back.
        LOOK = 6
        SCORE_AT = {0: 0, 3: 1, 6: 2, 8: 3, 11: 4, 14: 5}
        vtp = []
        for g in range(NPAIR):
            t = pvt.tile([P, 2, C], FP8, tag=f"vt{g}")
            vtp.append(t)
        for g in range(NPAIR):
            if g in SCORE_AT:
                i = SCORE_AT[g]
                se_q[seq[i]] = emit_scores(*seq[i])
            pool, tag = ((ps_feat, "feat"), (ps_sums, "sums"))[g % 2]
            # one [128,512] bank holds both halves of the pair; a single
            # accumulation group spans all four matmuls, and ONE wide copy
            # produces the fp8 DoubleRow lhsT tile.
            vp = pool.tile([P, 2, C], F32, tag=tag, name=f"vp{g}")
            for i in range(2):
                mt = 2 * g + i
                for kc in range(KC):
                    nc.tensor.matmul(
                        vp[:, i:i + 1, :],
                        lhsT=r_sb[kc][mt // 8][:, (mt % 8) * P:(mt % 8 + 1) * P],
                        rhs=wv_t[kc][:],
                        start=(i == 0 and kc == 0),
                        stop=(i == 1 and kc == KC - 1),
                    )
            if g < 8:
                nc.scalar.copy(vtp[g][:], vp[:])
            else:
                nc.vector.tensor_copy(vtp[g][:], vp[:])

        # ---- main attention loop (scores/exp LOOK pairs ahead) ----------
        fc = sm = None
        for idx, (nt, g) in enumerate(seq):
            if g == 0:
                fc = [ps_feat.tile([P, NT], F32, tag="feat", name=f"fc{nt}_{i}")
                      for i in range(2)]
                sm = ps_sums.tile([P, NT], F32, tag="sums")
            if idx + LOOK < len(seq):
                se_q[seq[idx + LOOK]] = emit_scores(*seq[idx + LOOK])
            se = se_q.pop((nt, g))
            first = g == 0
            last = g == NPAIR - 1
            for h in range(2):
                nc.tensor.matmul(
                    fc[h][:],
                    lhsT=vtp[g][:, :, h * P:(h + 1) * P],
                    rhs=se[:],
                    start=first, stop=last,
                    perf_mode=DR,
                )
            nc.tensor.matmul(
                sm[:], lhsT=ones2[:], rhs=se[:],
                start=first, stop=last,
                perf_mode=DR,
            )
            if last:
                n0 = nt * NT
                rcb = pout.tile([P, NT], F32, tag="rcb")
                nc.vector.reciprocal_approx_fast(out=rcb[:], in_=sm[:])
                # All four mults first: they are what release the fc PSUM
                # banks for the next nt's feat accumulation. Adds + DMAs after.
                HNT = NT // 2
                tmps = []
                for c in range(2):
                    for hh in range(2):
                        s = slice(hh * HNT, (hh + 1) * HNT)
                        tmp = pout.tile([P, HNT], F32, tag=f"tmp{c}{hh}",
                                        name=f"tmp{nt}_{c}_{hh}")
                        nc.vector.tensor_tensor(tmp[:], fc[c][:, s], rcb[:, s],
                                                OP.mult)
                        tmps.append((c, hh, tmp))
                for c, hh, tmp in tmps:
                    ot = pout.tile([P, HNT], F32, tag=f"ot{c}{hh}",
                                   name=f"ot{nt}_{c}_{hh}")
                    nc.vector.tensor_scalar(ot[:], tmp[:], bv_t[c], None,
                                            OP.add)
                    nc.sync.dma_start(
                        out[c * P:(c + 1) * P, n0 + hh * HNT:n0 + (hh + 1) * HNT],
                        ot[:])


_BUILT = None


def _build():
    global _BUILT
    if _BUILT is not None:
        return _BUILT
    nc = bacc.Bacc("TRN2", target_bir_lowering=False, debug=False)
    io = {
        # Declaration order == host->HBM upload order: small tensors first,
        # then d (needed earliest), then r.
        "biasv": nc.dram_tensor("biasv", [P, 4], F32, kind="ExternalInput"),
        "wqt4": nc.dram_tensor("wqt4", [C, P], BF16, kind="ExternalInput"),
        "wkt4": nc.dram_tensor("wkt4", [C, P], BF16, kind="ExternalInput"),
        "wvt": nc.dram_tensor("wvt", [C, C], BF16, kind="ExternalInput"),
        "d": nc.dram_tensor("d", [HW // 4, HW // 4], BF16, kind="ExternalInput"),
        "r": nc.dram_tensor("r", [HW // 4, HW // 4], BF16, kind="ExternalInput"),
        "out": nc.dram_tensor("out", [C, NQ], F32, kind="ExternalOutput"),
    }
    with tile.TileContext(nc) as tc:
        _emit(tc, io)
    nc.compile()
    _BUILT = nc
    return nc


def _in_maps(rgb, depth, Wq, bq, Wk, bk, Wv, bv):
    f = np.float32
    bf = ml_dtypes.bfloat16
    d_all = np.ascontiguousarray(depth.reshape(B, C, HW)).astype(bf)
    r_all = np.ascontiguousarray(rgb.reshape(B, C, HW)).astype(bf)
    wqt4 = np.ascontiguousarray(np.tile(np.asarray(Wq, f).T, (1, 4))).astype(bf)
    wkt4 = np.ascontiguousarray(np.tile(np.asarray(Wk, f).T, (1, 4))).astype(bf)
    wvt = np.ascontiguousarray(np.asarray(Wv, f).T).astype(bf)
    biasv = np.stack([np.tile(np.asarray(bq, f), 4),
                      np.tile(np.asarray(bk, f), 4),
                      np.asarray(bv, f)[:P],
                      np.asarray(bv, f)[P:]], axis=1)
    biasv = np.ascontiguousarray(biasv, dtype=f)
    def _pack(a):
        # [256, 4096] -> [1024, 1024]: row block ch*2+kc holds
        # a[kc*128:(kc+1)*128, ch*1024:(ch+1)*1024] (contiguous 256KB tiles,
        # upload order == consumption order).
        v = a.reshape(KC, P, 4, 1024)
        return np.ascontiguousarray(v.transpose(2, 0, 1, 3).reshape(1024, 1024))

    maps = []
    for core in range(8):
        b, half = core // 2, core % 2
        if half == 0:
            d_c, r_c = d_all[b], r_all[b]
        else:
            # Rotate so this core's query tokens are the first NQ columns;
            # key/value column order is a free permutation of the reduction.
            d_c = np.roll(d_all[b], -NQ, axis=1)
            r_c = np.roll(r_all[b], -NQ, axis=1)
        maps.append({
            "d": _pack(d_c),
            "r": _pack(r_c),
            "wqt4": wqt4, "wkt4": wkt4, "wvt": wvt,
            "biasv": biasv,
        })
    return maps


def kernel(rgb, depth, Wq, bq, Wk, bk, Wv, bv, **run_kwargs):
    nc = _build()
    maps = _in_maps(rgb, depth, Wq, bq, Wk, bk, Wv, bv)
    res = run_bass_kernel_spmd(nc, maps, core_ids=list(range(8)), **run_kwargs)
    results = res.results if hasattr(res, "results") else res
    out = np.empty((B, C, HW), dtype=np.float32)
    for core in range(8):
        b, half = core // 2, core % 2
        out[b][:, half * NQ:(half + 1) * NQ] = results[core]["out"]
    kernel.last_results = res
    return out.reshape(B, C, H, W)


# revision 43
# speedup vs baseline: 1.0137x; 1.0137x over previous
"""CrossAttention (DFFNet) Trainium2 Bass kernel.

Shapes (hardcoded): rgb/depth [4, 256, 64, 64] f32; Wq/Wk [32, 256]; Wv [256, 256].

    q = Wq @ d + bq          [B, 32, 4096]
    k = Wk @ d + bk          [B, 32, 4096]
    v = Wv @ r + bv          [B, 256, 4096]
    scores = q^T k           [B, 4096, 4096], softmax over keys (last dim)
    feat = v @ mask^T        [B, 256, 4096]

Sharding: 8 cores = 4 batches x 2 query-halves (2048 queries each). Each core
gets full depth/rgb for its batch (keys/values span all 4096 tokens) plus its
query-half slice of depth.

Device layout: scores are computed TRANSPOSED, st[m, n] (keys m on partitions,
queries n free) so the feat matmul needs no transposes:
  - v^T[m, c] is produced directly by  r-slice^T @ Wv^T.
  - feat[c, n] = sum_m v^T[m, c] * exp(st[m, n]) / S[n].
  - softmax denominator S[n] = sum_m exp(st[m,n]) via ones-lhsT matmul.
The PE array is the bottleneck (99% busy), so the exp tiles and v^T tiles are
stored as fp8-e4m3 and the feat + sums matmuls run in DoubleRow perf mode
(two 128-deep key tiles contracted per instruction at 0.5 cycles/row), halving
their PE streaming time. exp is computed with a built-in bias of -ln(8)
(out = exp(st)/8) to center values in fp8e4's range [2^-9, 240]; the scale
cancels exactly in the softmax normalization since the denominator uses the
same scaled tiles. Scores stay bf16 (K=32, output-bound, fp8 wouldn't help).

Inputs arrive from the host pre-cast to bf16 (halves DMA bytes and removes
all on-chip f32->bf16 staging casts). Normalization: 1/S via fast reciprocal
(f32), cast to fp16, broadcast to 128 partitions via a K=1 fp16 matmul.
"""

import math

import numpy as np
import ml_dtypes

import concourse.bacc as bacc
import concourse.bass as bass
import concourse.mybir as mybir
import concourse.tile as tile
from concourse.bass_utils import run_bass_kernel_spmd

B, C, H, W = 4, 256, 64, 64
HW = H * W            # 4096
CQK = 32
P = 128
NQ = HW // 2          # 2048 queries per core
NT = 512              # query tile
N_NT = NQ // NT       # 4
MT = HW // P          # 32 key tiles
NPAIR = MT // 2       # 16 DoubleRow key-tile pairs
KC = C // P           # 2 contraction tiles for the projections

F32 = mybir.dt.float32
F16 = mybir.dt.float16
BF16 = mybir.dt.bfloat16
FP8 = mybir.dt.float8e4
AF = mybir.ActivationFunctionType
OP = mybir.AluOpType
DR = mybir.MatmulPerfMode.DoubleRow

EXP_BIAS = -math.log(8.0)   # exp(st)/8: keeps weights in fp8e4 normal range


def _emit(tc, io):
    nc = tc.nc
    # d and r arrive packed chunk-major with the two 128-row contraction
    # blocks interleaved: row block b = ch*2 + kc holds
    # orig[kc*128:(kc+1)*128, ch*1024:(ch+1)*1024]. Each SBUF tile is one
    # contiguous 256KB DRAM block, and the host->HBM upload order matches
    # the order the projections consume tiles in.
    d = io["d"].ap()          # [1024, 1024] bf16 depth (query-half rotated)
    r = io["r"].ap()          # [1024, 1024] bf16 rgb, same packing/rotation
    wqt4 = io["wqt4"].ap()    # [256, 128] bf16 = tile(Wq.T, (1,4))
    wkt4 = io["wkt4"].ap()    # [256, 128] bf16
    wvt = io["wvt"].ap()      # [256, 256] bf16 = Wv.T
    # all bias vectors packed in one tensor so ONE early DMA covers them:
    # col0 = tile(bq,4), col1 = tile(bk,4), col2/3 = bv[0:128]/bv[128:256]
    biasv = io["biasv"].ap()  # [128, 4] f32
    out = io["out"].ap()      # [256, 2048] f32

    from contextlib import ExitStack

    with ExitStack() as ctx:
        pw = ctx.enter_context(tc.tile_pool(name="weights", bufs=1))
        pin = ctx.enter_context(tc.tile_pool(name="inputs", bufs=1))
        pqk = ctx.enter_context(tc.tile_pool(name="qk", bufs=1))
        pvt = ctx.enter_context(tc.tile_pool(name="vt", bufs=1))
        pse = ctx.enter_context(tc.tile_pool(name="stexp", bufs=8))
        psmall = ctx.enter_context(tc.tile_pool(name="small", bufs=2))
        pout = ctx.enter_context(tc.tile_pool(name="outsb", bufs=4))
        ps_st = ctx.enter_context(
            tc.tile_pool(name="ps_st", bufs=2, space=bass.MemorySpace.PSUM))
        ps_feat = ctx.enter_context(
            tc.tile_pool(name="ps_feat", bufs=2, space=bass.MemorySpace.PSUM))
        ps_sums = ctx.enter_context(
            tc.tile_pool(name="ps_sums", bufs=2, space=bass.MemorySpace.PSUM))

        # ---- constants / weights (inputs already bf16: direct DMA) -----
        # The packed bias vector goes FIRST on the sync ring: it is consumed
        # by the earliest DVE ops, and issuing it late would alias its
        # completion semaphore with later big transfers (false dependency).
        bias_sb = pw.tile([P, 4], F32, tag="biasv")
        nc.sync.dma_start(bias_sb[:], biasv[:])
        bq_sb = bias_sb[:, 0:1]
        bk_sb = bias_sb[:, 1:2]
        bv_t = [bias_sb[:, 2:3], bias_sb[:, 3:4]]
        wq_t, wk_t, wv_t = [], [], []
        for kc in range(KC):
            t = pw.tile([P, P], BF16, tag=f"wq{kc}")
            nc.sync.dma_start(t[:], wqt4[kc * P:(kc + 1) * P, :])
            wq_t.append(t)
            t = pw.tile([P, P], BF16, tag=f"wk{kc}")
            nc.sync.dma_start(t[:], wkt4[kc * P:(kc + 1) * P, :])
            wk_t.append(t)
            t = pw.tile([P, C], BF16, tag=f"wv{kc}")
            nc.sync.dma_start(t[:], wvt[kc * P:(kc + 1) * P, :])
            wv_t.append(t)
        # Full-width ones lhsT: dual-fp8 LDWEIGHTS requires col_grp=0xf (all
        # 128 PE columns), so a [128,2,1] ones vector is illegal. The [128,2,128]
        # form also lands S[n] pre-broadcast on all 128 PSUM partitions, which
        # removes the separate K=1 broadcast matmul from the critical path.
        ones2 = pw.tile([P, 2, P], FP8, tag="ones2")
        nc.vector.memset(ones2[:], 1.0)
        ebias = pw.tile([P, 1], F32, tag="ebias")
        nc.vector.memset(ebias[:], EXP_BIAS)

        # ---- inputs (per-chunk tiles so projections start early) -------
        # d and r are column-rotated per core on the host so this core's
        # query tokens are d's FIRST 2048 columns (q-proj reads d chunks 0-1
        # directly; no separate dq input). Key/value order is a free
        # permutation: softmax and feat both just sum over keys.
        # DMA rings: d + r[kc1] on gpsimd (fast ring), r[kc0] on sync after
        # the weights. The scalar/Activation queue stays clear of
        # descriptors so exp activations are never queued behind DMA.
        def _tile_of(dram_ap, kc, ch, pref, eng):
            t = pin.tile([P, 1024], BF16, tag=f"{pref}{kc}_{ch}",
                         name=f"{pref}{kc}_{ch}")
            b = ch * 2 + kc
            eng.dma_start(t[:], dram_ap[b * P:(b + 1) * P, :])
            return t

        d_sb = [[None] * 4 for _ in range(KC)]
        for ch in range(4):
            for kc in range(KC):
                d_sb[kc][ch] = _tile_of(d, kc, ch, "d", nc.gpsimd)
        r_sb = [[None] * 4 for _ in range(KC)]
        for ch in range(4):
            for kc in range(KC):
                r_sb[kc][ch] = _tile_of(r, kc, ch, "r", nc.sync)

        # ---- q / k projections (4x-replicated layouts for row packing) --
        # q4h/k4q are SEPARATE tiles per 1024-column block so a score matmul
        # on block b only depends on block b's single wide bias-add, not on
        # the whole serialized bias chain (subtile deps are conservative).
        # Emission interleaves q-half0 / k-qtr0 FIRST: the first score matmul
        # needs exactly those two bias-adds, so the serial DVE chain reaches
        # them before anything else.
        q4h = [pqk.tile([P, 1024], BF16, tag=f"q4_{h}", name=f"q4_{h}")
               for h in range(2)]
        k4q = [pqk.tile([P, 1024], BF16, tag=f"k4_{q}", name=f"k4_{q}")
               for q in range(4)]

        # Projection PSUM tiles live in the fc/sums banks (idle until the
        # main loop) so the score pipeline's two stp buffers never chain
        # behind the serial DVE bias-add sequence. One [128,512] tile per
        # 512-column sub-block (same 1-bank slot size as the fc/sums tags).
        def _proj(dst, blk, sub, w_t, b_sb, pool, tag, pref):
            pp = pool.tile([P, NT], F32, tag=tag, name=f"{pref}{blk}_{sub}")
            for kc in range(KC):
                nc.tensor.matmul(
                    pp[:],
                    lhsT=w_t[kc][:],
                    rhs=d_sb[kc][blk][:, sub * NT:(sub + 1) * NT],
                    start=(kc == 0),
                    stop=(kc == KC - 1),
                )
            nc.vector.tensor_scalar(
                dst[:, sub * NT:(sub + 1) * NT], pp[:], b_sb, None, OP.add)

        for blk, sub in [(0, 0), (0, 1), (1, 0), (1, 1)]:
            _proj(q4h[blk], blk, sub, wq_t, bq_sb, ps_feat, "feat", "qp")
            _proj(k4q[blk], blk, sub, wk_t, bk_sb, ps_sums, "sums", "kp")
        for blk, sub in [(2, 0), (2, 1), (3, 0), (3, 1)]:
            _proj(k4q[blk], blk, sub, wk_t, bk_sb, ps_sums, "sums", "kp")

        def emit_scores(nt, g):
            stp = ps_st.tile([P, 2, NT], F32, tag="stp", name=f"stp{nt}_{g}")
            for j in range(2):
                mt = 2 * g + j
                nc.tensor.matmul(
                    stp[:, j:j + 1, :],
                    lhsT=k4q[mt // 8][32 * j:32 * j + 32,
                                      (mt % 8) * P:(mt % 8 + 1) * P],
                    rhs=q4h[nt // 2][32 * j:32 * j + 32,
                                     (nt % 2) * NT:(nt % 2 + 1) * NT],
                    start=True,
                    stop=True,
                    tile_position=(32 * j, 0),
                )
            se = pse.tile([P, 2, NT], FP8, tag="se", name=f"se{nt}_{g}")
            nc.scalar.activation(se[:], stp[:], AF.Exp, bias=ebias[:])
            return se

        seq = [(nt, g) for nt in range(N_NT) for g in range(NPAIR)]
        se_q = {}

        # ---- bank phase: v^T projection interleaved with the first LOOK
        # score/exp pairs. vtp[g][p, i, c] = v[c, (2g+i)*128 + p], stored fp8
        # in DoubleRow pair layout (no bias; bias added at the end). The vp
        # PSUM tiles rotate through the fc + sums banks (idle until the main
        # loop). Casts for the first half go to ACT (interleaving between the
        # banked exps); the second half to DVE (after the k/q bias adds) so
        # both finish just before the fc/sm allocations need the banks back.
        LOOK = 6
        SCORE_AT = {0: 0, 3: 1, 6: 2, 8: 3, 11: 4, 14: 5}
        vtp = []
        for g in range(NPAIR):
            t = pvt.tile([P, 2, C], FP8, tag=f"vt{g}")
            vtp.append(t)
        for g in range(NPAIR):
            if g in SCORE_AT:
                i = SCORE_AT[g]
                se_q[seq[i]] = emit_scores(*seq[i])
            pool, tag = ((ps_feat, "feat"), (ps_sums, "sums"))[g % 2]
            # one [128,512] bank holds both halves of the pair; a single
            # accumulation group spans all four matmuls, and ONE wide copy
            # produces the fp8 DoubleRow lhsT tile.
            vp = pool.tile([P, 2, C], F32, tag=tag, name=f"vp{g}")
            for i in range(2):
                mt = 2 * g + i
                for kc in range(KC):
                    nc.tensor.matmul(
                        vp[:, i:i + 1, :],
                        lhsT=r_sb[kc][mt // 8][:, (mt % 8) * P:(mt % 8 + 1) * P],
                        rhs=wv_t[kc][:],
                        start=(i == 0 and kc == 0),
                        stop=(i == 1 and kc == KC - 1),
                    )
            if g < 8:
                nc.scalar.copy(vtp[g][:], vp[:])
            else:
                nc.vector.tensor_copy(vtp[g][:], vp[:])

        # ---- main attention loop (scores/exp LOOK pairs ahead) ----------
        fc = sm = None
        for idx, (nt, g) in enumerate(seq):
            if g == 0:
                fc = [ps_feat.tile([P, NT], F32, tag="feat", name=f"fc{nt}_{i}")
                      for i in range(2)]
                sm = ps_sums.tile([P, NT], F32, tag="sums")
            if idx + LOOK < len(seq):
                se_q[seq[idx + LOOK]] = emit_scores(*seq[idx + LOOK])
            se = se_q.pop((nt, g))
            first = g == 0
            last = g == NPAIR - 1
            for h in range(2):
                nc.tensor.matmul(
                    fc[h][:],
                    lhsT=vtp[g][:, :, h * P:(h + 1) * P],
                    rhs=se[:],
                    start=first, stop=last,
                    perf_mode=DR,
                )
            nc.tensor.matmul(
                sm[:], lhsT=ones2[:], rhs=se[:],
                start=first, stop=last,
                perf_mode=DR,
            )
            if last:
                n0 = nt * NT
                rcb = pout.tile([P, NT], F32, tag="rcb")
                nc.vector.reciprocal_approx_fast(out=rcb[:], in_=sm[:])
                # All four mults first: they are what release the fc PSUM
                # banks for the next nt's feat accumulation. Adds + DMAs after.
                HNT = NT // 2
                tmps = []
                for c in range(2):
                    for hh in range(2):
                        s = slice(hh * HNT, (hh + 1) * HNT)
                        tmp = pout.tile([P, HNT], F32, tag=f"tmp{c}{hh}",
                                        name=f"tmp{nt}_{c}_{hh}")
                        nc.vector.tensor_tensor(tmp[:], fc[c][:, s], rcb[:, s],
                                                OP.mult)
                        tmps.append((c, hh, tmp))
                for c, hh, tmp in tmps:
                    ot = pout.tile([P, HNT], F32, tag=f"ot{c}{hh}",
                                   name=f"ot{nt}_{c}_{hh}")
                    nc.vector.tensor_scalar(ot[:], tmp[:], bv_t[c], None,
                                            OP.add)
                    nc.sync.dma_start(
                        out[c * P:(c + 1) * P, n0 + hh * HNT:n0 + (hh + 1) * HNT],
                        ot[:])


_BUILT = None


def _build():
    global _BUILT
    if _BUILT is not None:
        return _BUILT
    nc = bacc.Bacc("TRN2", target_bir_lowering=False, debug=False)
    io = {
        # Declaration order == host->HBM upload order: small tensors first,
        # then d (needed earliest), then r.
        "biasv": nc.dram_tensor("biasv", [P, 4], F32, kind="ExternalInput"),
        "wqt4": nc.dram_tensor("wqt4", [C, P], BF16, kind="ExternalInput"),
        "wkt4": nc.dram_tensor("wkt4", [C, P], BF16, kind="ExternalInput"),
        "wvt": nc.dram_tensor("wvt", [C, C], BF16, kind="ExternalInput"),
        "d": nc.dram_tensor("d", [HW // 4, HW // 4], BF16, kind="ExternalInput"),
        "r": nc.dram_tensor("r", [HW // 4, HW // 4], BF16, kind="ExternalInput"),
        "out": nc.dram_tensor("out", [C, NQ], F32, kind="ExternalOutput"),
    }
    with tile.TileContext(nc) as tc:
        _emit(tc, io)
    nc.compile()
    _BUILT = nc
    return nc


def _in_maps(rgb, depth, Wq, bq, Wk, bk, Wv, bv):
    f = np.float32
    bf = ml_dtypes.bfloat16
    d_all = np.ascontiguousarray(depth.reshape(B, C, HW)).astype(bf)
    r_all = np.ascontiguousarray(rgb.reshape(B, C, HW)).astype(bf)
    wqt4 = np.ascontiguousarray(np.tile(np.asarray(Wq, f).T, (1, 4))).astype(bf)
    wkt4 = np.ascontiguousarray(np.tile(np.asarray(Wk, f).T, (1, 4))).astype(bf)
    wvt = np.ascontiguousarray(np.asarray(Wv, f).T).astype(bf)
    biasv = np.stack([np.tile(np.asarray(bq, f), 4),
                      np.tile(np.asarray(bk, f), 4),
                      np.asarray(bv, f)[:P],
                      np.asarray(bv, f)[P:]], axis=1)
    biasv = np.ascontiguousarray(biasv, dtype=f)
    def _pack(a):
        # [256, 4096] -> [1024, 1024]: row block ch*2+kc holds
        # a[kc*128:(kc+1)*128, ch*1024:(ch+1)*1024] (contiguous 256KB tiles,
        # upload order == consumption order).
        v = a.reshape(KC, P, 4, 1024)
        return np.ascontiguousarray(v.transpose(2, 0, 1, 3).reshape(1024, 1024))

    maps = []
    for core in range(8):
        b, half = core // 2, core % 2
        if half == 0:
            d_c, r_c = d_all[b], r_all[b]
        else:
            # Rotate so this core's query tokens are the first NQ columns;
            # key/value column order is a free permutation of the reduction.
            d_c = np.roll(d_all[b], -NQ, axis=1)
            r_c = np.roll(r_all[b], -NQ, axis=1)
        maps.append({
            "d": _pack(d_c),
            "r": _pack(r_c),
            "wqt4": wqt4, "wkt4": wkt4, "wvt": wvt,
            "biasv": biasv,
        })
    return maps


def kernel(rgb, depth, Wq, bq, Wk, bk, Wv, bv, **run_kwargs):
    nc = _build()
    maps = _in_maps(rgb, depth, Wq, bq, Wk, bk, Wv, bv)
    res = run_bass_kernel_spmd(nc, maps, core_ids=list(range(8)), **run_kwargs)
    results = res.results if hasattr(res, "results") else res
    out = np.empty((B, C, HW), dtype=np.float32)
    for core in range(8):
        b, half = core // 2, core % 2
        out[b][:, half * NQ:(half + 1) * NQ] = results[core]["out"]
    kernel.last_results = res
    return out.reshape(B, C, H, W)


# revision 44
# speedup vs baseline: 1.0340x; 1.0200x over previous
"""CrossAttention (DFFNet) Trainium2 Bass kernel.

Shapes (hardcoded): rgb/depth [4, 256, 64, 64] f32; Wq/Wk [32, 256]; Wv [256, 256].

    q = Wq @ d + bq          [B, 32, 4096]
    k = Wk @ d + bk          [B, 32, 4096]
    v = Wv @ r + bv          [B, 256, 4096]
    scores = q^T k           [B, 4096, 4096], softmax over keys (last dim)
    feat = v @ mask^T        [B, 256, 4096]

Sharding: 8 cores = 4 batches x 2 query-halves (2048 queries each). Each core
gets full depth/rgb for its batch (keys/values span all 4096 tokens) plus its
query-half slice of depth.

Device layout: scores are computed TRANSPOSED, st[m, n] (keys m on partitions,
queries n free) so the feat matmul needs no transposes:
  - v^T[m, c] is produced directly by  r-slice^T @ Wv^T.
  - feat[c, n] = sum_m v^T[m, c] * exp(st[m, n]) / S[n].
  - softmax denominator S[n] = sum_m exp(st[m,n]) via ones-lhsT matmul.
The PE array is the bottleneck (99% busy), so the exp tiles and v^T tiles are
stored as fp8-e4m3 and the feat + sums matmuls run in DoubleRow perf mode
(two 128-deep key tiles contracted per instruction at 0.5 cycles/row), halving
their PE streaming time. exp is computed with a built-in bias of -ln(8)
(out = exp(st)/8) to center values in fp8e4's range [2^-9, 240]; the scale
cancels exactly in the softmax normalization since the denominator uses the
same scaled tiles. Scores stay bf16 (K=32, output-bound, fp8 wouldn't help).

Inputs arrive from the host pre-cast to bf16 (halves DMA bytes and removes
all on-chip f32->bf16 staging casts). Normalization: 1/S via fast reciprocal
(f32), cast to fp16, broadcast to 128 partitions via a K=1 fp16 matmul.
"""

import math

import numpy as np
import ml_dtypes

import concourse.bacc as bacc
import concourse.bass as bass
import concourse.mybir as mybir
import concourse.tile as tile
from concourse.bass_utils import run_bass_kernel_spmd

B, C, H, W = 4, 256, 64, 64
HW = H * W            # 4096
CQK = 32
P = 128
NQ = HW // 2          # 2048 queries per core
NT = 512              # query tile
N_NT = NQ // NT       # 4
MT = HW // P          # 32 key tiles
NPAIR = MT // 2       # 16 DoubleRow key-tile pairs
KC = C // P           # 2 contraction tiles for the projections

F32 = mybir.dt.float32
F16 = mybir.dt.float16
BF16 = mybir.dt.bfloat16
FP8 = mybir.dt.float8e4
AF = mybir.ActivationFunctionType
OP = mybir.AluOpType
DR = mybir.MatmulPerfMode.DoubleRow

EXP_BIAS = -math.log(8.0)   # exp(st)/8: keeps weights in fp8e4 normal range


def _emit(tc, io):
    nc = tc.nc
    # d and r arrive packed chunk-major with the two 128-row contraction
    # blocks interleaved: row block b = ch*2 + kc holds
    # orig[kc*128:(kc+1)*128, ch*1024:(ch+1)*1024]. Each SBUF tile is one
    # contiguous 256KB DRAM block, and the host->HBM upload order matches
    # the order the projections consume tiles in.
    d = io["d"].ap()          # [1024, 1024] bf16 depth (query-half rotated)
    r = io["r"].ap()          # [1024, 1024] bf16 rgb, same packing/rotation
    wqt4 = io["wqt4"].ap()    # [256, 128] bf16 = tile(Wq.T, (1,4))
    wkt4 = io["wkt4"].ap()    # [256, 128] bf16
    wvt = io["wvt"].ap()      # [256, 256] bf16 = Wv.T
    # all bias vectors packed in one tensor so ONE early DMA covers them:
    # col0 = tile(bq,4), col1 = tile(bk,4), col2/3 = bv[0:128]/bv[128:256]
    biasv = io["biasv"].ap()  # [128, 4] f32
    out = io["out"].ap()      # [256, 2048] f32

    from contextlib import ExitStack

    with ExitStack() as ctx:
        pw = ctx.enter_context(tc.tile_pool(name="weights", bufs=1))
        pin = ctx.enter_context(tc.tile_pool(name="inputs", bufs=1))
        pqk = ctx.enter_context(tc.tile_pool(name="qk", bufs=1))
        pvt = ctx.enter_context(tc.tile_pool(name="vt", bufs=1))
        pse = ctx.enter_context(tc.tile_pool(name="stexp", bufs=8))
        psmall = ctx.enter_context(tc.tile_pool(name="small", bufs=2))
        pout = ctx.enter_context(tc.tile_pool(name="outsb", bufs=4))
        ps_st = ctx.enter_context(
            tc.tile_pool(name="ps_st", bufs=2, space=bass.MemorySpace.PSUM))
        ps_feat = ctx.enter_context(
            tc.tile_pool(name="ps_feat", bufs=2, space=bass.MemorySpace.PSUM))
        ps_sums = ctx.enter_context(
            tc.tile_pool(name="ps_sums", bufs=2, space=bass.MemorySpace.PSUM))

        # ---- constants / weights (inputs already bf16: direct DMA) -----
        # The packed bias vector goes FIRST on the sync ring: it is consumed
        # by the earliest DVE ops, and issuing it late would alias its
        # completion semaphore with later big transfers (false dependency).
        bias_sb = pw.tile([P, 4], F32, tag="biasv")
        nc.sync.dma_start(bias_sb[:], biasv[:])
        bq_sb = bias_sb[:, 0:1]
        bk_sb = bias_sb[:, 1:2]
        bv_t = [bias_sb[:, 2:3], bias_sb[:, 3:4]]
        wq_t, wk_t, wv_t = [], [], []
        for kc in range(KC):
            t = pw.tile([P, P], BF16, tag=f"wq{kc}")
            nc.sync.dma_start(t[:], wqt4[kc * P:(kc + 1) * P, :])
            wq_t.append(t)
            t = pw.tile([P, P], BF16, tag=f"wk{kc}")
            nc.sync.dma_start(t[:], wkt4[kc * P:(kc + 1) * P, :])
            wk_t.append(t)
            t = pw.tile([P, C], BF16, tag=f"wv{kc}")
            nc.sync.dma_start(t[:], wvt[kc * P:(kc + 1) * P, :])
            wv_t.append(t)
        # Full-width ones lhsT: dual-fp8 LDWEIGHTS requires col_grp=0xf (all
        # 128 PE columns), so a [128,2,1] ones vector is illegal. The [128,2,128]
        # form also lands S[n] pre-broadcast on all 128 PSUM partitions, which
        # removes the separate K=1 broadcast matmul from the critical path.
        ones2 = pw.tile([P, 2, P], FP8, tag="ones2")
        nc.vector.memset(ones2[:], 1.0)
        ebias = pw.tile([P, 1], F32, tag="ebias")
        nc.vector.memset(ebias[:], EXP_BIAS)

        # ---- PE warm-up ------------------------------------------------
        # PE_HAM only releases the 2.4GHz clock after ~3.4us of sustained
        # matmul activity; the real projections start DMA-gated and gappy,
        # which left the whole bank phase at 1.2GHz. Burn ~6us of dummy
        # matmuls (no data deps beyond the memsets) so the array is warm
        # when the first projection issues.
        wtile = pw.tile([P, NT], FP8, tag="wtile")
        nc.vector.memset(wtile[:], 1.0)
        warm = ps_st.tile([P, 2, NT], F32, tag="stp", name="warm")
        for w in range(20):
            nc.tensor.matmul(
                warm[:, w % 2:w % 2 + 1, :],
                lhsT=ones2[:, 0, :],
                rhs=wtile[:],
                start=True,
                stop=True,
            )

        # ---- inputs (per-chunk tiles so projections start early) -------
        # d and r are column-rotated per core on the host so this core's
        # query tokens are d's FIRST 2048 columns (q-proj reads d chunks 0-1
        # directly; no separate dq input). Key/value order is a free
        # permutation: softmax and feat both just sum over keys.
        # DMA rings: d + r[kc1] on gpsimd (fast ring), r[kc0] on sync after
        # the weights. The scalar/Activation queue stays clear of
        # descriptors so exp activations are never queued behind DMA.
        def _tile_of(dram_ap, kc, ch, pref, eng):
            t = pin.tile([P, 1024], BF16, tag=f"{pref}{kc}_{ch}",
                         name=f"{pref}{kc}_{ch}")
            b = ch * 2 + kc
            eng.dma_start(t[:], dram_ap[b * P:(b + 1) * P, :])
            return t

        d_sb = [[None] * 4 for _ in range(KC)]
        for ch in range(4):
            for kc in range(KC):
                d_sb[kc][ch] = _tile_of(d, kc, ch, "d", nc.gpsimd)
        r_sb = [[None] * 4 for _ in range(KC)]
        for ch in range(4):
            for kc in range(KC):
                r_sb[kc][ch] = _tile_of(r, kc, ch, "r", nc.sync)

        # ---- q / k projections (4x-replicated layouts for row packing) --
        # q4h/k4q are SEPARATE tiles per 1024-column block so a score matmul
        # on block b only depends on block b's single wide bias-add, not on
        # the whole serialized bias chain (subtile deps are conservative).
        # Emission interleaves q-half0 / k-qtr0 FIRST: the first score matmul
        # needs exactly those two bias-adds, so the serial DVE chain reaches
        # them before anything else.
        q4h = [pqk.tile([P, 1024], BF16, tag=f"q4_{h}", name=f"q4_{h}")
               for h in range(2)]
        k4q = [pqk.tile([P, 1024], BF16, tag=f"k4_{q}", name=f"k4_{q}")
               for q in range(4)]

        # Projection PSUM tiles live in the fc/sums banks (idle until the
        # main loop) so the score pipeline's two stp buffers never chain
        # behind the serial DVE bias-add sequence. One [128,512] tile per
        # 512-column sub-block (same 1-bank slot size as the fc/sums tags).
        def _proj(dst, blk, sub, w_t, b_sb, pool, tag, pref):
            pp = pool.tile([P, NT], F32, tag=tag, name=f"{pref}{blk}_{sub}")
            for kc in range(KC):
                nc.tensor.matmul(
                    pp[:],
                    lhsT=w_t[kc][:],
                    rhs=d_sb[kc][blk][:, sub * NT:(sub + 1) * NT],
                    start=(kc == 0),
                    stop=(kc == KC - 1),
                )
            nc.vector.tensor_scalar(
                dst[:, sub * NT:(sub + 1) * NT], pp[:], b_sb, None, OP.add)

        for blk, sub in [(0, 0), (0, 1), (1, 0), (1, 1)]:
            _proj(q4h[blk], blk, sub, wq_t, bq_sb, ps_feat, "feat", "qp")
            _proj(k4q[blk], blk, sub, wk_t, bk_sb, ps_sums, "sums", "kp")
        for blk, sub in [(2, 0), (2, 1), (3, 0), (3, 1)]:
            _proj(k4q[blk], blk, sub, wk_t, bk_sb, ps_sums, "sums", "kp")

        def emit_scores(nt, g):
            stp = ps_st.tile([P, 2, NT], F32, tag="stp", name=f"stp{nt}_{g}")
            for j in range(2):
                mt = 2 * g + j
                nc.tensor.matmul(
                    stp[:, j:j + 1, :],
                    lhsT=k4q[mt // 8][32 * j:32 * j + 32,
                                      (mt % 8) * P:(mt % 8 + 1) * P],
                    rhs=q4h[nt // 2][32 * j:32 * j + 32,
                                     (nt % 2) * NT:(nt % 2 + 1) * NT],
                    start=True,
                    stop=True,
                    tile_position=(32 * j, 0),
                )
            se = pse.tile([P, 2, NT], FP8, tag="se", name=f"se{nt}_{g}")
            nc.scalar.activation(se[:], stp[:], AF.Exp, bias=ebias[:])
            return se

        seq = [(nt, g) for nt in range(N_NT) for g in range(NPAIR)]
        se_q = {}

        # ---- bank phase: v^T projection interleaved with the first LOOK
        # score/exp pairs. vtp[g][p, i, c] = v[c, (2g+i)*128 + p], stored fp8
        # in DoubleRow pair layout (no bias; bias added at the end). The vp
        # PSUM tiles rotate through the fc + sums banks (idle until the main
        # loop). Casts for the first half go to ACT (interleaving between the
        # banked exps); the second half to DVE (after the k/q bias adds) so
        # both finish just before the fc/sm allocations need the banks back.
        LOOK = 6
        SCORE_AT = {0: 0, 3: 1, 6: 2, 8: 3, 11: 4, 14: 5}
        vtp = []
        for g in range(NPAIR):
            t = pvt.tile([P, 2, C], FP8, tag=f"vt{g}")
            vtp.append(t)
        for g in range(NPAIR):
            if g in SCORE_AT:
                i = SCORE_AT[g]
                se_q[seq[i]] = emit_scores(*seq[i])
            pool, tag = ((ps_feat, "feat"), (ps_sums, "sums"))[g % 2]
            # one [128,512] bank holds both halves of the pair; a single
            # accumulation group spans all four matmuls, and ONE wide copy
            # produces the fp8 DoubleRow lhsT tile.
            vp = pool.tile([P, 2, C], F32, tag=tag, name=f"vp{g}")
            for i in range(2):
                mt = 2 * g + i
                for kc in range(KC):
                    nc.tensor.matmul(
                        vp[:, i:i + 1, :],
                        lhsT=r_sb[kc][mt // 8][:, (mt % 8) * P:(mt % 8 + 1) * P],
                        rhs=wv_t[kc][:],
                        start=(i == 0 and kc == 0),
                        stop=(i == 1 and kc == KC - 1),
                    )
            if g < 8:
                nc.scalar.copy(vtp[g][:], vp[:])
            else:
                nc.vector.tensor_copy(vtp[g][:], vp[:])

        # ---- main attention loop (scores/exp LOOK pairs ahead) ----------
        fc = sm = None
        for idx, (nt, g) in enumerate(seq):
            if g == 0:
                fc = [ps_feat.tile([P, NT], F32, tag="feat", name=f"fc{nt}_{i}")
                      for i in range(2)]
                sm = ps_sums.tile([P, NT], F32, tag="sums")
            if idx + LOOK < len(seq):
                se_q[seq[idx + LOOK]] = emit_scores(*seq[idx + LOOK])
            se = se_q.pop((nt, g))
            first = g == 0
            last = g == NPAIR - 1
            for h in range(2):
                nc.tensor.matmul(
                    fc[h][:],
                    lhsT=vtp[g][:, :, h * P:(h + 1) * P],
                    rhs=se[:],
                    start=first, stop=last,
                    perf_mode=DR,
                )
            nc.tensor.matmul(
                sm[:], lhsT=ones2[:], rhs=se[:],
                start=first, stop=last,
                perf_mode=DR,
            )
            if last:
                n0 = nt * NT
                rcb = pout.tile([P, NT], F32, tag="rcb")
                nc.vector.reciprocal_approx_fast(out=rcb[:], in_=sm[:])
                # All four mults first: they are what release the fc PSUM
                # banks for the next nt's feat accumulation. Adds + DMAs after.
                HNT = NT // 2
                tmps = []
                for c in range(2):
                    for hh in range(2):
                        s = slice(hh * HNT, (hh + 1) * HNT)
                        tmp = pout.tile([P, HNT], F32, tag=f"tmp{c}{hh}",
                                        name=f"tmp{nt}_{c}_{hh}")
                        nc.vector.tensor_tensor(tmp[:], fc[c][:, s], rcb[:, s],
                                                OP.mult)
                        tmps.append((c, hh, tmp))
                for c, hh, tmp in tmps:
                    ot = pout.tile([P, HNT], F32, tag=f"ot{c}{hh}",
                                   name=f"ot{nt}_{c}_{hh}")
                    nc.vector.tensor_scalar(ot[:], tmp[:], bv_t[c], None,
                                            OP.add)
                    nc.sync.dma_start(
                        out[c * P:(c + 1) * P, n0 + hh * HNT:n0 + (hh + 1) * HNT],
                        ot[:])


_BUILT = None


def _build():
    global _BUILT
    if _BUILT is not None:
        return _BUILT
    nc = bacc.Bacc("TRN2", target_bir_lowering=False, debug=False)
    io = {
        # Declaration order == host->HBM upload order: small tensors first,
        # then d (needed earliest), then r.
        "biasv": nc.dram_tensor("biasv", [P, 4], F32, kind="ExternalInput"),
        "wqt4": nc.dram_tensor("wqt4", [C, P], BF16, kind="ExternalInput"),
        "wkt4": nc.dram_tensor("wkt4", [C, P], BF16, kind="ExternalInput"),
        "wvt": nc.dram_tensor("wvt", [C, C], BF16, kind="ExternalInput"),
        "d": nc.dram_tensor("d", [HW // 4, HW // 4], BF16, kind="ExternalInput"),
        "r": nc.dram_tensor("r", [HW // 4, HW // 4], BF16, kind="ExternalInput"),
        "out": nc.dram_tensor("out", [C, NQ], F32, kind="ExternalOutput"),
    }
    with tile.TileContext(nc) as tc:
        _emit(tc, io)
    nc.compile()
    _BUILT = nc
    return nc


def _in_maps(rgb, depth, Wq, bq, Wk, bk, Wv, bv):
    f = np.float32
    bf = ml_dtypes.bfloat16
    d_all = np.ascontiguousarray(depth.reshape(B, C, HW)).astype(bf)
    r_all = np.ascontiguousarray(rgb.reshape(B, C, HW)).astype(bf)
    wqt4 = np.ascontiguousarray(np.tile(np.asarray(Wq, f).T, (1, 4))).astype(bf)
    wkt4 = np.ascontiguousarray(np.tile(np.asarray(Wk, f).T, (1, 4))).astype(bf)
    wvt = np.ascontiguousarray(np.asarray(Wv, f).T).astype(bf)
    biasv = np.stack([np.tile(np.asarray(bq, f), 4),
                      np.tile(np.asarray(bk, f), 4),
                      np.asarray(bv, f)[:P],
                      np.asarray(bv, f)[P:]], axis=1)
    biasv = np.ascontiguousarray(biasv, dtype=f)
    def _pack(a):
        # [256, 4096] -> [1024, 1024]: row block ch*2+kc holds
        # a[kc*128:(kc+1)*128, ch*1024:(ch+1)*1024] (contiguous 256KB tiles,
        # upload order == consumption order).
        v = a.reshape(KC, P, 4, 1024)
        return np.ascontiguousarray(v.transpose(2, 0, 1, 3).reshape(1024, 1024))

    maps = []
    for core in range(8):
        b, half = core // 2, core % 2
        if half == 0:
            d_c, r_c = d_all[b], r_all[b]
        else:
            # Rotate so this core's query tokens are the first NQ columns;
            # key/value column order is a free permutation of the reduction.
            d_c = np.roll(d_all[b], -NQ, axis=1)
            r_c = np.roll(r_all[b], -NQ, axis=1)
        maps.append({
            "d": _pack(d_c),
            "r": _pack(r_c),
            "wqt4": wqt4, "wkt4": wkt4, "wvt": wvt,
            "biasv": biasv,
        })
    return maps


def kernel(rgb, depth, Wq, bq, Wk, bk, Wv, bv, **run_kwargs):
    nc = _build()
    maps = _in_maps(rgb, depth, Wq, bq, Wk, bk, Wv, bv)
    res = run_bass_kernel_spmd(nc, maps, core_ids=list(range(8)), **run_kwargs)
    results = res.results if hasattr(res, "results") else res
    out = np.empty((B, C, HW), dtype=np.float32)
    for core in range(8):
        b, half = core // 2, core % 2
        out[b][:, half * NQ:(half + 1) * NQ] = results[core]["out"]
    kernel.last_results = res
    return out.reshape(B, C, H, W)
